# revision 93
# baseline (speedup 1.0000x reference)
"""CIN (Compressed Interaction Network) forward kernel for 8 Trainium2 NeuronCores.

Reference computation (per batch b, embedding dim d):
    x0 = inputs[b, :, d]                 # [F=39]
    h0 = x0
    for k in 0..2:
        z  = outer(x0, h_{k})            # [F * Hk]
        h_{k+1} = z @ Wk + bk            # [256]
    out[b] = concat_k sum_d h_{k+1}      # [768]

Strategy: data-parallel over batch (64 per core).  Per core, rows r = (b, d)
are 2048 GEMM rows.  Layers 0 and 1 run transposed (x0T[f, r], hT[u, r]):
the Khatri-Rao product z_T[(i,j), r] = x0T[i, r] * hT[j, r] is materialized
k-tile by k-tile on the Vector engine (fp16 -> 2x mode) and consumed by the
Tensor engine as the moving operand of [K,512]-shaped matmuls into PSUM.

Layer 2 is NOT computed per-(b,d).  Its pooled output only needs
    out2[b,u] = sum_{i,j} W2[(i,j),u] * M2[b,i,j],
    M2[b,i,j] = sum_d x[b,i,d] * h2[b,j,d],
so instead of the 10.5 GFLOP full layer-2 GEMM we do:
  1. h2 [256, 2048] is transposed to h2t [r, u] by XBAR DMA transposes
     (no Tensor-engine time).
  2. M2 per 4-batch group g (128 rows): stationary = h2t chunk [128 r, 128 j],
     moving = block-diagonal X-tile [128 r, 160 (i,s)] (host-prepped; includes
     a ones-row i=39 whose columns yield out1 = sum_d h2 for free).
  3. out2 = 78-k-tile GEMM: stationary = M2 slices [128 j, 64 b], moving =
     W2 tiles [128, 256], accumulating into one [64, 256] PSUM region.
This replaces ~165us of layer-2 matmul with ~20us.

The d-sum for layer 0 is a Vector-engine reduce of the fp16 h1 tiles;
out1 comes from the M2 ones-row (fp32, copied straight from PSUM).
Biases: b0/b1 are added on-device during PSUM evacuation (they feed the
recurrence / M2); the D*b2 contribution to out2 is added host-side (exact).
"""

import os
import sys

import numpy as np

for _p in ("/opt/trn_rl_repo", "/root/.axon_site/_ro/trn_rl_repo"):
    if os.path.isdir(_p) and _p not in sys.path:
        sys.path.insert(0, _p)

N_CORES = 8
B, F, D = 512, 39, 32
U = 256
BL = B // N_CORES          # 64 batches per core
R = BL * D                 # 2048 GEMM rows per core
NB = 512                   # matmul moving free-dim (one PSUM bank of fp32)
NRB = R // NB              # 4 row blocks
NP0 = F * (F + 1) // 2     # 780 symmetric (i <= j) pairs for layer 0
KT0S = 7                   # layer-0 k-tiles of 128 pairs (last holds 12)
FP = 42                    # replication factor in the x0r DRAM layout
K12 = F * U                # 9984
KT12 = K12 // 128          # 78 k-tiles; kt = (i, half)
G = 16                     # 4-batch groups (128 rows each)
IW = F + 1                 # i-values in the M2 X-tile (39 + ones row)
N_DUMMY = 20               # PE keep-warm matmuls across the L1 -> M2 gap
N_WARM = 8                 # PE warm-up matmuls at kernel start

DT = "float16"             # device compute dtype for z / W / h ("float16" | "bfloat16")

_prog_cache = {}


def _np_dt():
    import ml_dtypes

    return np.float16 if DT == "float16" else ml_dtypes.bfloat16


def _build_program():
    import concourse.mybir as mybir
    from concourse import bacc, tile

    dt = mybir.dt
    cdt = getattr(dt, DT)
    f32 = dt.float32

    nc = bacc.Bacc(
        "TRN2", target_bir_lowering=False, debug=False, num_devices=N_CORES
    )
    # x0/S zero-padded to 128 rows on the host: the broadcast matmuls
    # contract over all 128 partitions so the HAM activity monitor sees a
    # fully-busy PE (K=39 matmuls keep the clock gate at 1.2 GHz).
    x0s_p = nc.declare_dram_parameter("x0s", [128, R], cdt, isOutput=False)
    # x0 rows each replicated 42x in DRAM: broadcast DMAs read distinct
    # addresses (HBM bank spread) instead of hammering one 4KB row.
    x0r_p = nc.declare_dram_parameter("x0r", [F * FP, R], cdt, isOutput=False)
    w0_p = nc.declare_dram_parameter("w0", [128, KT0S, U], cdt, isOutput=False)
    s_p = nc.declare_dram_parameter("s", [128, KT0S, 128], cdt, isOutput=False)
    sj_p = nc.declare_dram_parameter("sj", [128, KT0S, 128], cdt, isOutput=False)
    w1_p = nc.declare_dram_parameter("w1", [128, KT12, U], cdt, isOutput=False)
    w2_p = nc.declare_dram_parameter("w2", [128, KT12, U], cdt, isOutput=False)
    bias_p = nc.declare_dram_parameter("bias", [128, 4], f32, isOutput=False)
    xblk_p = nc.declare_dram_parameter("xblk", [128, G, 4 * IW], cdt, isOutput=False)
    ident_p = nc.declare_dram_parameter("ident", [128, 128], cdt, isOutput=False)
    out01_p = nc.declare_dram_parameter("out01", [128, 4, BL], f32, isOutput=True)
    out2_p = nc.declare_dram_parameter("out2", [BL, U], f32, isOutput=True)

    with tile.TileContext(nc) as tc:
        with (
            tc.tile_pool(name="const", bufs=1) as constp,
            tc.tile_pool(name="wpool", bufs=1) as wpool,
            tc.tile_pool(name="xb", bufs=5) as xbp,
            tc.tile_pool(name="zp", bufs=4) as zp,
            tc.tile_pool(name="hp", bufs=1) as hp,
            tc.tile_pool(name="psum", bufs=1, space="PSUM") as psp,
        ):
            # broadcast DMAs source from DRAM (re-reading one SBUF partition
            # 128x serializes on its port) and alternate trigger engines so
            # both dynamic HW queues run in parallel.
            bcast_n = [0]

            def bcast(dst, src_ap):
                eng = nc.sync if bcast_n[0] % 2 == 0 else nc.scalar
                bcast_n[0] += 1
                eng.dma_start(dst, src_ap)

            out_sb = constp.tile([128, 4, BL], f32, tag="out")
            h_tiles = {
                (l, c): hp.tile([128, R], cdt, tag=f"h{l}{c}", name=f"h{l}{c}")
                for l in range(2)
                for c in range(2)
            }
            h2t = {
                c: constp.tile([128, G, 128], cdt, tag=f"h2t{c}", name=f"h2t{c}")
                for c in range(2)
            }
            m2t = {
                jh: constp.tile([128, IW, BL], cdt, tag=f"m2t{jh}", name=f"m2t{jh}")
                for jh in range(2)
            }
            out2_sb = constp.tile([BL, U], f32, tag="out2sb")
            xblk = constp.tile([128, G, 4 * IW], cdt, tag="xblk")
            ident = constp.tile([128, 128], cdt, tag="ident")

            # all 8 PSUM banks, shared by every phase
            ps_all = {
                (c, r): psp.tile(
                    [128, NB], f32, tag=f"ps_{c}_{r}", name=f"ps_{c}_{r}"
                )
                for c in range(2)
                for r in range(NRB)
            }
            pslist = [ps_all[(c, r)] for c in range(2) for r in range(NRB)]

            # ---- prologue.  Layer 0 needs almost no HBM traffic: x0 itself,
            # the one-hot broadcast selectors S_i/S_j, and the folded
            # symmetric W0.  Both z operands (xpi, xpj) are generated
            # on-chip by PE broadcast matmuls - no replicated-pair DMA.
            x0sb = constp.tile([128, R], cdt, tag="x0sb")
            s_sb = constp.tile([128, KT0S, 128], cdt, tag="s_sb")
            sj_sb = constp.tile([128, KT0S, 128], cdt, tag="sj_sb")
            w0 = wpool.tile([128, KT0S, U], cdt, tag="w0")
            w1 = wpool.tile([128, KT12, U], cdt, tag="w1")
            bias = constp.tile([128, 4], f32, tag="bias")

            # All layer-0-critical tensors ride the gpsimd software-DGE
            # queue: it dispatches immediately and sustains ~150GB/s while
            # the HW dynamic queues ramp slowly (~55GB/s early).  The HW
            # queues carry only the zero pad half of x0 plus layer-1
            # prefetches.  W1 is fully preloaded before layer 1: its k-tiles
            # are matmul stationaries, and a mid-layer chunk arriving late
            # is read STALE by the LDWEIGHTS pull-ahead (no PSUM is free
            # during layer 1 for a guard matmul); only the first chunk needs
            # a boundary guard, the rest have tens of us of margin.
            nc.gpsimd.dma_start(x0sb[0:64, :], x0s_p[0:64, :])
            # first two k-tiles of the selectors/weights ride the scalar
            # queue so layer 0 can start while the bulk streams on gpsimd;
            # the warm-up guards cover these first chunks, and the bulk
            # arrives with >5us margin before its first k-tile use.
            nc.scalar.dma_start(s_sb[:, 0:2, :], s_p[:, 0:2, :])
            nc.scalar.dma_start(sj_sb[:, 0:2, :], sj_p[:, 0:2, :])
            nc.scalar.dma_start(w0[:, 0:2, :], w0_p[:, 0:2, :])
            nc.gpsimd.dma_start(s_sb[:, 2:, :], s_p[:, 2:, :])
            nc.gpsimd.dma_start(sj_sb[:, 2:, :], sj_p[:, 2:, :])
            nc.gpsimd.dma_start(w0[:, 2:, :], w0_p[:, 2:, :])
            nc.sync.dma_start(x0sb[64:128, :], x0s_p[64:128, :])
            nc.scalar.dma_start(bias[:, :], bias_p[:, :])
            nc.gpsimd.dma_start(w1[:, 0:13, :], w1_p[:, 0:13, :])
            nc.gpsimd.dma_start(w1[:, 13:46, :], w1_p[:, 13:46, :])
            nc.gpsimd.dma_start(w1[:, 46:78, :], w1_p[:, 46:78, :])
            nc.gpsimd.dma_start(ident[:, :], ident_p[:, :])

            # ---- PE warm-up: the HAM clock gate needs ~3.4us of sustained
            # matmul activity to unthrottle 1.2 -> 2.4 GHz; a few dummy
            # matmuls bridge the initial DMA latency.
            warm_ps = pslist[6]
            for _ in range(N_WARM):
                nc.tensor.matmul(
                    warm_ps[:, :],
                    h_tiles[(0, 0)][:, :128],
                    h_tiles[(0, 0)][:, :NB],
                    start=True,
                    stop=True,
                )
            if N_WARM:
                # stationary-guard matmuls: LDWEIGHTS carries no semaphore
                # wait and dispatches right after the previous matmul, so a
                # just-DMA'd stationary can be read stale.  A matmul whose
                # MOVING operand covers the fresh region forces the wait
                # before any later LDWEIGHTS can dispatch.
                nc.tensor.matmul(
                    warm_ps[:, 0:384],
                    h_tiles[(0, 0)][:, :128],
                    s_sb[:, 0:3, :],
                    start=True,
                    stop=True,
                )
                nc.tensor.matmul(
                    warm_ps[:, 0:384],
                    h_tiles[(0, 0)][:, :128],
                    sj_sb[:, 0:3, :],
                    start=True,
                    stop=True,
                )
                nc.tensor.matmul(
                    warm_ps[:, :],
                    h_tiles[(0, 0)][:, :128],
                    w0[:, 0:2, :],
                    start=True,
                    stop=True,
                )

            def make_x(i, nm, eng=None):
                t = xbp.tile([128, R], cdt, tag="xi", name=nm, bufs=8)
                src = (
                    x0r_p[i * FP : i * FP + 32, :]
                    .unsqueeze(1)
                    .to_broadcast((32, 4, R))
                )
                if eng is None:
                    bcast(t[:, :], src)
                else:
                    eng.dma_start(t[:, :], src)
                return t

            l1_pre = {i: make_x(i, f"l1x{i}") for i in (0, 1)}

            def evac(l, c, r, ps_t):
                # PSUM -> SBUF fp16 with per-partition bias; c=0 on DVE,
                # c=1 on the otherwise-idle Scalar engine.
                if c == 0:
                    nc.vector.tensor_scalar_add(
                        h_tiles[(l, c)][:, r * NB : (r + 1) * NB],
                        ps_t[:, :],
                        bias[:, l * 2 + c : l * 2 + c + 1],
                    )
                else:
                    nc.scalar.activation(
                        h_tiles[(l, c)][:, r * NB : (r + 1) * NB],
                        ps_t[:, :],
                        mybir.ActivationFunctionType.Identity,
                        bias=bias[:, l * 2 + c : l * 2 + c + 1],
                    )

            def do_layer(l, w_t, z_fn, kt_n, kt_hook=None):
                ps = [[ps_all[(c, r)] for r in range(NRB)] for c in range(2)]
                for kt in range(kt_n):
                    if kt_hook is not None:
                        kt_hook(kt)
                    klen, z_t = z_fn(kt)
                    for c in range(2):
                        lhsT = w_t[:klen, kt, c * 128 : (c + 1) * 128]
                        for r in range(NRB):
                            nc.tensor.matmul(
                                ps[c][r][:, :],
                                lhsT,
                                z_t[:klen, r * NB : (r + 1) * NB],
                                start=(kt == 0),
                                stop=(kt == kt_n - 1),
                            )
                # evacuations gate the next phase and free the PSUM banks.
                # Layer 1 goes r-descending: the XBAR transposes + M2 matmuls
                # consume high row blocks first (g descending).
                rord = range(NRB) if l == 0 else range(NRB - 1, -1, -1)
                for c in range(2):
                    for r in rord:
                        evac(l, c, r, ps[c][r])

            def h_reduce_piece(l, c, q):
                # quarter-sized d-sum chunks so the DVE never blocks the
                # z-production tensor-tensor stream for more than ~600ns
                nc.vector.tensor_reduce(
                    out_sb[:, l * 2 + c, 16 * q : 16 * q + 16],
                    h_tiles[(l, c)].rearrange("p (b d) -> p b d", d=D)[
                        :, 16 * q : 16 * q + 16, :
                    ],
                    axis=mybir.AxisListType.X,
                    op=mybir.AluOpType.add,
                )

            # ---- layer 0 (symmetric): k = (i <= j) pairs, W0 folded so each
            # unordered pair appears once (780 rows -> 7 k-tiles).  Per
            # (k-tile, row-block): xpi and xpj are one-hot matmul broadcasts
            # of x0 rows into PSUM (no DMA), xpj is evacuated to fp16 SBUF,
            # z = xpi * xpj on the DVE, then two accumulating matmuls.
            # Two r-phases share the 8 PSUM banks: h1 accum in P0-P3 (both
            # phases; phase-1 starts after the phase-0 evacuations), xpi
            # rotation in P4/P5, xpj rotation in P6/P7.
            xpi_rot = [pslist[4], pslist[5]]
            xpj_rot = [pslist[6], pslist[7]]
            hmap = {(0, 0): pslist[0], (0, 1): pslist[1],
                    (1, 0): pslist[2], (1, 1): pslist[3]}
            xpn = [0]
            for ph in range(2):
                for kt in range(KT0S):
                    klen = NP0 - 128 * kt if kt == KT0S - 1 else 128
                    for q in range(2):
                        rb = 2 * ph + q
                        xp = xpi_rot[xpn[0] % 2]
                        xq = xpj_rot[xpn[0] % 2]
                        xpn[0] += 1
                        nc.tensor.matmul(
                            xp[:klen, :],
                            s_sb[:, kt, :klen],
                            x0sb[:, rb * NB : (rb + 1) * NB],
                            start=True,
                            stop=True,
                        )
                        nc.tensor.matmul(
                            xq[:klen, :],
                            sj_sb[:, kt, :klen],
                            x0sb[:, rb * NB : (rb + 1) * NB],
                            start=True,
                            stop=True,
                        )
                        # only one TT operand may live in PSUM: evacuate xpj
                        # to fp16 SBUF on the otherwise-idle Scalar engine
                        xj_t = zp.tile([128, NB], cdt, tag="xj", name="xj0", bufs=3)
                        nc.scalar.activation(
                            xj_t[:klen, :],
                            xq[:klen, :],
                            mybir.ActivationFunctionType.Identity,
                        )
                        z_t = zp.tile([128, NB], cdt, tag="z", name="z0")
                        nc.vector.tensor_mul(
                            z_t[:klen, :],
                            xp[:klen, :],
                            xj_t[:klen, :],
                        )
                        for c in range(2):
                            nc.tensor.matmul(
                                hmap[(c, q)][:, :],
                                w0[:klen, kt, c * 128 : (c + 1) * 128],
                                z_t[:klen, :],
                                start=(kt == 0),
                                stop=(kt == KT0S - 1),
                            )
                for q in range(2):
                    for c in range(2):
                        evac(0, c, 2 * ph + q, hmap[(c, q)])

            # ---- layer 1: z[(i, j), r] = x0[i, r] * h1[j, r], k = i*256 + j ----
            def z_layer1(premade):
                xcur = [None]

                def fn(kt):
                    i, half = kt // 2, kt % 2
                    if half == 0:
                        if i in premade:
                            xcur[0] = premade[i]
                        else:
                            # the last few xi broadcasts ride the gpsimd
                            # software-DGE queue so the sync/scalar HW queues
                            # drain before the layer-end h2 transposes
                            xcur[0] = make_x(
                                i, "xi", eng=nc.gpsimd if i >= 33 else None
                            )
                    z_t = zp.tile([128, R], cdt, tag="z")
                    if kt < 2:
                        # boundary pipelining: slice-wise TT so each matmul's z
                        # slice is ready right after its h evacuation lands
                        for r in range(NRB):
                            nc.vector.tensor_mul(
                                z_t[:, r * NB : (r + 1) * NB],
                                xcur[0][:, r * NB : (r + 1) * NB],
                                h_tiles[(0, half)][:, r * NB : (r + 1) * NB],
                            )
                    else:
                        nc.vector.tensor_mul(
                            z_t[:, :], xcur[0][:, :], h_tiles[(0, half)][:, :]
                        )
                    return 128, z_t

                return fn

            w2 = wpool.tile([128, KT12, U], cdt, tag="w2")

            # stream W2 at spread points in layer 1 (it is only a MOVING
            # operand at the end, so hook-paced arrival is race-free)
            w_sched = {26: (w2, w2_p, 0), 34: (w2, w2_p, 1),
                       42: (w2, w2_p, 2), 50: (w2, w2_p, 3), 58: (w2, w2_p, 4),
                       64: (w2, w2_p, 5)}

            # deferred layer-0 d-sum in 8 quarter-chunks, off the boundary
            # path and small enough not to starve z production
            red_sched = {4: (0, 0), 6: (1, 0), 9: (0, 1), 11: (1, 1),
                         14: (0, 2), 16: (1, 2), 19: (0, 3), 21: (1, 3)}

            def w_hook(kt):
                if kt in red_sched:
                    c, q = red_sched[kt]
                    h_reduce_piece(0, c, q)
                if kt == 24:
                    nc.sync.dma_start(out01_p[:, 0:2, :], out_sb[:, 0:2, :])
                if kt == 30:
                    nc.gpsimd.dma_start(xblk[:, :, :], xblk_p[:, :, :])
                if kt in w_sched:
                    # hook-paced on sync/scalar: the gpsimd queue would fire
                    # these immediately (no deps) and steal early bandwidth
                    wt, wp, c = w_sched[kt]
                    lo = 13 * c
                    (nc.sync if c % 2 else nc.scalar).dma_start(
                        wt[:, lo : lo + 13, :], wp[:, lo : lo + 13, :]
                    )

            # boundary guard: force the first W1 chunk's completion wait onto
            # the PE stream before layer 1's first LDWEIGHTS; the garbage it
            # writes is cleared by the kt=0 start=True matmuls.
            nc.tensor.matmul(
                pslist[0][:, 0:256],
                x0sb[:, 0:128],
                w1[:, 0, :],
                start=True,
                stop=True,
            )
            do_layer(1, w1, z_layer1(l1_pre), KT12, kt_hook=w_hook)

            # ---- layer-2 output path ----
            # All PSUM here is allocated as fresh pool tiles (rotating on
            # the layer-1 tags) so the WAR/WAW ordering vs the layer-1
            # accumulators is tracked by the pool.
            tags7 = [f"ps_{c}_{r}" for c in range(2) for r in range(NRB)][:7]

            def scratch(n):
                return psp.tile([128, n], f32, tag="ps_1_3", name="scr")

            def dummy():
                # keep the PE warm (HAM MID window is ~3.4us); garbage
                # matmul, only waits on the (1,3) evacuation via rotation
                nc.tensor.matmul(
                    scratch(256),
                    h_tiles[(0, 0)][:, :128],
                    h_tiles[(0, 0)][:, :256],
                    start=True,
                    stop=True,
                )

            def guard(rhs, npsum):
                # dispatch-ordered wait for a freshly written region (see the
                # warm-up guards): reads it as the moving operand of a matmul
                nc.tensor.matmul(
                    scratch(npsum),
                    xblk[:, 0, 0:128],
                    rhs,
                    start=True,
                    stop=True,
                )

            if N_DUMMY:
                for _ in range(N_DUMMY):
                    dummy()

            # M2[b,i,j] = sum_d x[b,i,d] h2[b,j,d] per 4-batch group:
            # stationary = h2t chunk [128 r, 128 j], moving = block-diag
            # X-tile [128 r, 160 (i,s)].  Column i=39 is the ones-row -> out1.
            # Transposes run as REGULAR matmuls against identity (out =
            # lhsT.T @ I, exact): unlike transpose-mode these pipeline
            # (~110ns) and count as PE-busy for the HAM clock gate.  The
            # loop is software-pipelined one row-block deep so block rb-1's
            # transposes overlap block rb's copies and M2 matmuls.
            def trans_block(rb):
                guard(h_tiles[(1, 1)][:, rb * NB : (rb + 1) * NB], NB)
                guard(h_tiles[(1, 0)][:, rb * NB : (rb + 1) * NB], NB)
                for c in range(2):
                    tp = psp.tile(
                        [128, 4, 128], f32, tag=f"ps_{c}_{rb}", name="tps"
                    )
                    for k in range(4):
                        nc.tensor.matmul(
                            tp[:, k, :],
                            h_tiles[(1, c)][
                                :, rb * NB + 128 * k : rb * NB + 128 * (k + 1)
                            ],
                            ident[:, :],
                            start=True,
                            stop=True,
                        )
                    if c == 0:
                        nc.vector.tensor_copy(
                            h2t[c][:, 4 * rb : 4 * rb + 4, :], tp[:, :, :]
                        )
                    else:
                        nc.scalar.activation(
                            h2t[c][:, 4 * rb : 4 * rb + 4, :],
                            tp[:, :, :],
                            mybir.ActivationFunctionType.Identity,
                        )

            trans_block(3)
            m2n = 0
            for g in range(G - 1, -1, -1):
                if g % 4 == 3:
                    rb = g // 4
                    if rb > 0:
                        trans_block(rb - 1)
                    guard(h2t[1][:, 4 * rb : 4 * rb + 4, :], NB)
                    guard(h2t[0][:, 4 * rb : 4 * rb + 4, :], NB)
                for jh in range(2):
                    pm = psp.tile(
                        [128, 160], f32, tag=tags7[(2 * g + jh) % 7], name="m2ps"
                    )
                    nc.tensor.matmul(
                        pm[:, :],
                        h2t[jh][:, g, :],
                        xblk[:, g, :],
                        start=True,
                        stop=True,
                    )
                    if m2n % 2 == 0:
                        nc.vector.tensor_copy(
                            m2t[jh][:, :, 4 * g : 4 * g + 4],
                            pm[:, :].rearrange("p (i s) -> p i s", s=4),
                        )
                        nc.vector.tensor_copy(
                            out_sb[:, 2 + jh, 4 * g : 4 * g + 4],
                            pm[:, 156:160],
                        )
                    else:
                        nc.scalar.activation(
                            m2t[jh][:, :, 4 * g : 4 * g + 4],
                            pm[:, :].rearrange("p (i s) -> p i s", s=4),
                            mybir.ActivationFunctionType.Identity,
                        )
                        nc.scalar.activation(
                            out_sb[:, 2 + jh, 4 * g : 4 * g + 4],
                            pm[:, 156:160],
                            mybir.ActivationFunctionType.Identity,
                        )
                    m2n += 1

            # out2[b,u] = sum over 78 k-tiles: stationary = M2 slice
            # [128 j, 64 b], moving = W2 tile [128, 256].  Guards force the
            # M2-evacuation waits before the first stationary loads.
            guard(m2t[0][:, 0, :], BL)
            guard(m2t[1][:, 0, :], BL)
            o2ps = psp.tile([BL, 256], f32, tag="ps_1_3", name="o2ps")
            for kt in range(KT12):
                i, jh = divmod(kt, 2)
                nc.tensor.matmul(
                    o2ps[:, :],
                    m2t[jh][:, i, :],
                    w2[:, kt, :],
                    start=(kt == 0),
                    stop=(kt == KT12 - 1),
                )

            nc.vector.tensor_copy(out2_sb[:, :], o2ps[:, :])
            nc.sync.dma_start(out01_p[:, 2:4, :], out_sb[:, 2:4, :])
            nc.scalar.dma_start(out2_p[:, :], out2_sb[:, :])

    nc.compile()
    return nc


def _get_program():
    if "nc" not in _prog_cache:
        _prog_cache["nc"] = _build_program()
    return _prog_cache["nc"]


def _prep_maps(inputs):
    cdt = _np_dt()
    x = np.asarray(inputs["inputs"], np.float32)          # [512, 39, 32]
    Ws = [np.asarray(inputs[f"W{k}"], np.float32) for k in range(3)]
    bs = [np.asarray(inputs[f"b{k}"], np.float32) for k in range(3)]

    # layer-0 symmetric fold: unordered pair (i <= j) -> one k-row carrying
    # W0[i,j] + W0[j,i] (just W0[i,i] on the diagonal), i-major enumeration.
    pairs = [(i, j) for i in range(F) for j in range(i, F)]
    W0m = Ws[0].reshape(F, F, U)
    w0t = np.zeros((KT0S, 128, U), np.float32)
    s_oh = np.zeros((128, KT0S, 128), np.float32)
    sj_oh = np.zeros((128, KT0S, 128), np.float32)
    for p, (i, j) in enumerate(pairs):
        w = W0m[i, j] + (W0m[j, i] if j > i else 0.0)
        w0t[p // 128, p % 128] = w
        s_oh[i, p // 128, p % 128] = 1.0
        sj_oh[j, p // 128, p % 128] = 1.0
    w_tiled = [
        w0t.transpose(1, 0, 2).astype(cdt),
        Ws[1].reshape(KT12, 128, U).transpose(1, 0, 2).astype(cdt),
        Ws[2].reshape(KT12, 128, U).transpose(1, 0, 2).astype(cdt),
    ]
    w_tiled = [np.ascontiguousarray(w) for w in w_tiled]
    s_oh = np.ascontiguousarray(s_oh.astype(cdt))
    sj_oh = np.ascontiguousarray(sj_oh.astype(cdt))
    bias = np.zeros((128, 4), np.float32)
    for l in range(2):
        for c in range(2):
            bias[:, l * 2 + c] = bs[l][c * 128 : (c + 1) * 128]
    ident = np.ascontiguousarray(np.eye(128, dtype=np.float32).astype(cdt))

    in_maps = []
    for core in range(N_CORES):
        xs = x[core * BL : (core + 1) * BL]               # [64, 39, 32]
        x0T = np.ascontiguousarray(
            xs.transpose(1, 0, 2).reshape(F, R).astype(cdt)
        )
        x0r = np.ascontiguousarray(np.repeat(x0T, FP, axis=0))
        x0pad = np.zeros((128, R), cdt)
        x0pad[:F] = x0T
        # block-diagonal X-tile for M2: xblk[s*32+d, g, i*4+s] = x[4g+s, i, d]
        # (i=39 -> 1.0); other s-slots zero so each group's contraction stays
        # within its own batch.
        xg = xs.reshape(G, 4, F, D)                       # [g, s, i, d]
        xblk = np.zeros((128, G, 4 * IW), np.float32)
        for s in range(4):
            blk = np.zeros((D, G, IW), np.float32)
            blk[:, :, :F] = xg[:, s].transpose(2, 0, 1)   # [d, g, i]
            blk[:, :, F] = 1.0
            xblk[s * D : (s + 1) * D, :, s :: 4] = blk
        in_maps.append(
            {
                "x0s": x0pad,
                "x0r": x0r,
                "w0": w_tiled[0],
                "w1": w_tiled[1],
                "w2": w_tiled[2],
                "bias": bias,
                "s": s_oh,
                "sj": sj_oh,
                "ident": ident,
                "xblk": np.ascontiguousarray(xblk.astype(cdt)),
            }
        )
    return in_maps, bs


def _finish_output(results, bs):
    outs = []
    for core in range(N_CORES):
        o = np.asarray(results[core]["out01"], np.float32)   # [128, 4, 64]
        o01 = o.transpose(2, 1, 0).reshape(BL, 2, U)         # [b, l, u]
        o2 = np.asarray(results[core]["out2"], np.float32)   # [64, 256]
        outs.append(
            np.concatenate([o01[:, 0, :], o01[:, 1, :], o2 + D * bs[2]], axis=1)
        )
    out = np.concatenate(outs, axis=0)
    return np.ascontiguousarray(out.astype(np.float32))


def kernel(**inputs) -> np.ndarray:
    from concourse.bass_utils import run_bass_kernel_spmd

    in_maps, bs = _prep_maps(inputs)
    nc = _get_program()
    res = run_bass_kernel_spmd(nc, in_maps, list(range(N_CORES))).results
    return _finish_output(res, bs)
